# revision 2
# baseline (speedup 1.0000x reference)
"""LoRALinear fused kernel for 8 trn2 NeuronCores — v4 (fp8 DoubleRow).

y = x @ (base + 2*(B@A))^T + bias,  x:[2,2048,4096], base:[4096,4096],
A:[8,4096], B:[4096,8], bias:[4096] -> y:[2,2048,4096], all fp32.

Sharding: 8-way token-parallel (512 tokens/core, replicated weights).

Numerics: host fuses W = base^T + 2*A^T B^T, then splits x*2^5 and
W*2^9 into fp8e4m3 hi/lo pairs (lo = fp8(residual), same scale — e4m3
has enough exponent range).  On device
  y*2^14 = xh@Wh + xl@Wh + xh@Wl + 128*fp8(bias*2^7)   (lo@lo dropped)
which measures 8.3e-4 absmax rel err (gate 2e-2).  All matmuls are fp8
MatmulPerfMode.DoubleRow: stationary [128,2,128], moving [128,2,512],
K=256 per MM at 0.5 cycles/row -> ~107ns/MM vs fp16's 213ns.  3 MMs
replace each fp16 MM pair-chunk = 0.75x fp16 cycles -> ~168us stream.

Layouts are host-pre-tiled so every DMA descriptor is >=1KB (fp8
halves element size; naive [D,O] slices would give 512B descriptors
and halve ring bandwidth): x as [128p, 32c, 512t], W as
[8ob, 128p, 32c, 512o].  Ring split: scalar={xh, Wl, bb}, sync=
{xl, Wh}; y evacs alternate rings.  Structure per core: 8 o-blocks of
512 douts; per block 4 token-tile PSUM accumulators (tags acc0-3 x
bufs=2), bias added via a DoubleRow close MM (ones-row ^ bias-row),
evac = DVE tensor_scalar_mul by 2^-14.  Last o-block runs token-outer
over resident W so the final drain is one split tile, not four.
"""
import sys

sys.path.insert(0, "/opt/trn_rl_repo")

import numpy as np

T_SH = 8                    # token shards (pure data-parallel)
T, D, O = 4096, 4096, 4096  # flattened tokens, d_in, d_out
TC = T // T_SH              # 512 tokens per core
KC = D // 128               # 32 contraction chunks
DK = KC // 2                # 16 DoubleRow pair-chunks
NB = O // 512               # 8 o-blocks of 512
TT = TC // 128              # 4 token tiles per core

XSC = 32.0                  # 2^5  x pre-scale
WSC = 512.0                 # 2^9  W pre-scale
BSC = 128.0                 # 2^7  bias pre-scale (ones row also 2^7)
OSC = 2.0 ** -14            # evac rescale

_cache = {}


def _build():
    import concourse.mybir as mybir
    import concourse.tile as tile
    from concourse import bacc

    f32 = mybir.dt.float32
    fp8 = mybir.dt.float8e4
    DR = mybir.MatmulPerfMode.DoubleRow

    nc = bacc.Bacc("TRN2", target_bir_lowering=False, debug=False,
                   num_devices=8)

    xh_d = nc.dram_tensor("xh", [128, KC, TC], fp8, kind="ExternalInput").ap()
    xl_d = nc.dram_tensor("xl", [128, KC, TC], fp8, kind="ExternalInput").ap()
    wh_d = nc.dram_tensor("wh", [NB, 128, KC, 512], fp8,
                          kind="ExternalInput").ap()
    wl_d = nc.dram_tensor("wl", [NB, 128, KC, 512], fp8,
                          kind="ExternalInput").ap()
    # bb[0,0,:] = fp8(bias*2^7), everything else zero
    bb_d = nc.dram_tensor("bb", [128, 2, O], fp8, kind="ExternalInput").ap()
    y_d = nc.dram_tensor("y", [TC, O], f32, kind="ExternalOutput").ap()

    # o-block DMA group layout, in dk (pair-chunk) units
    G_FIRST = [(0, 1), (1, 1), (2, 1), (3, 1),
               (4, 2), (6, 2), (8, 2), (10, 2), (12, 2), (14, 2)]
    G_STEADY = [(0, 1), (1, 1), (2, 2), (4, 2), (6, 2),
                (8, 2), (10, 2), (12, 2), (14, 2)]
    # x chunk-group slices (chunk units) interleaved into ob0's DMA stream
    X_GROUPS = [(0, 2), (2, 6), (6, 14), (14, 22), (22, 32)]

    with tile.TileContext(nc) as tc:
        with (
            tc.tile_pool(name="res", bufs=1) as res,
            tc.tile_pool(name="wst", bufs=8) as wst,
            tc.tile_pool(name="evac", bufs=6) as evac,
            tc.tile_pool(name="psum", bufs=1, space="PSUM") as psum,
        ):
            xh = res.tile([128, KC, TC], fp8)
            xl = res.tile([128, KC, TC], fp8)
            # ones stationary for the bias close: [0,0,:]=2^7
            ones_t = res.tile([128, 2, 128], fp8)
            nc.vector.memset(ones_t[:], 0.0)
            nc.vector.memset(ones_t[0:1, 0:1, :], BSC)
            bbt = res.tile([128, 2, O], fp8)

            ev_ring = [0]

            def close_and_evac(acc, t, ob, split_out=False):
                osl = slice(512 * ob, 512 * (ob + 1))
                nc.tensor.matmul(acc[:], ones_t[:], bbt[:, :, osl],
                                 start=False, stop=True, perf_mode=DR)
                ev = evac.tile([128, 512], f32, name=f"ev{t}", tag="ev")
                nc.vector.tensor_scalar_mul(ev[:], acc[:], OSC)
                tsl = slice(128 * t, 128 * (t + 1))
                if split_out:
                    nc.scalar.dma_start(y_d[tsl, osl.start:osl.start + 256],
                                        ev[:, 0:256])
                    nc.sync.dma_start(y_d[tsl, osl.start + 256:osl.stop],
                                      ev[:, 256:512])
                else:
                    ring = nc.scalar if ev_ring[0] % 2 == 0 else nc.sync
                    ev_ring[0] += 1
                    ring.dma_start(y_d[tsl, osl], ev[:])

            def dma_w_groups(ob, glist, interleave=None, tag_sfx=""):
                """Issue this o-block's W DMAs (hi on sync, lo on scalar),
                returning [(dk0, ndk, wht, wlt)]."""
                groups = []
                for g, (dk0, ndk) in enumerate(glist):
                    small = ndk == 1
                    wht = wst.tile([128, 2 * ndk, 512], fp8,
                                   name=f"wh{ob}_{g}",
                                   tag=("whs" if small else "wh") + tag_sfx,
                                   bufs=(4 if small and not tag_sfx else None))
                    wlt = wst.tile([128, 2 * ndk, 512], fp8,
                                   name=f"wl{ob}_{g}",
                                   tag=("wls" if small else "wl") + tag_sfx,
                                   bufs=(4 if small and not tag_sfx else None))
                    csl = slice(2 * dk0, 2 * (dk0 + ndk))
                    nc.sync.dma_start(wht[:], wh_d[ob][:, csl, :])
                    nc.scalar.dma_start(wlt[:], wl_d[ob][:, csl, :])
                    groups.append((dk0, ndk, wht, wlt))
                    if interleave is not None:
                        interleave(g)
                return groups

            def mm_block(accs, groups, ob):
                for dk0, ndk, wht, wlt in groups:
                    for j in range(ndk):
                        dk = dk0 + j
                        ks = slice(2 * dk, 2 * dk + 2)
                        js = slice(2 * j, 2 * j + 2)
                        for t in range(TT):
                            tsl = slice(128 * t, 128 * (t + 1))
                            nc.tensor.matmul(
                                accs[t][:], xh[:, ks, tsl], wht[:, js, :],
                                start=(dk == 0), stop=False, perf_mode=DR)
                            nc.tensor.matmul(
                                accs[t][:], xh[:, ks, tsl], wlt[:, js, :],
                                start=False, stop=False, perf_mode=DR)
                            nc.tensor.matmul(
                                accs[t][:], xl[:, ks, tsl], wht[:, js, :],
                                start=False, stop=False, perf_mode=DR)

            # ---- o-block 0: x streams interleaved with its W groups ----
            def x_filler(g):
                if g < len(X_GROUPS):
                    c0, c1 = X_GROUPS[g]
                    nc.scalar.dma_start(xh[:, c0:c1, :], xh_d[:, c0:c1, :])
                    nc.sync.dma_start(xl[:, c0:c1, :], xl_d[:, c0:c1, :])
                elif g == len(X_GROUPS):
                    # bias close data; not needed until first close ~45us in
                    nc.scalar.dma_start(bbt[:], bb_d[:])

            def o_block(ob, glist, interleave=None):
                accs = {
                    t: psum.tile([128, 512], f32, name=f"acc{t}_{ob}",
                                 tag=f"acc{t}", bufs=2)
                    for t in range(TT)
                }
                groups = dma_w_groups(ob, glist, interleave=interleave)
                mm_block(accs, groups, ob)
                for t in range(TT):
                    close_and_evac(accs[t], t, ob)

            o_block(0, G_FIRST, interleave=x_filler)
            for ob in range(1, NB - 2):
                o_block(ob, G_STEADY)

            # ---- last o-block: token-outer over resident W (prefetched
            # during block NB-2) so the 4 closes stagger ~5us apart ----
            wl_groups = []

            def prefetch_wlast(g):
                if g < len(G_STEADY):
                    wl_groups.extend(
                        dma_w_groups(NB - 1, [G_STEADY[g]], tag_sfx="L"))

            o_block(NB - 2, G_STEADY, interleave=prefetch_wlast)

            for t in range(TT):
                acc = psum.tile([128, 512], f32, name=f"acc{t}_last",
                                tag=f"acc{t}", bufs=2)
                tsl = slice(128 * t, 128 * (t + 1))
                for dk0, ndk, wht, wlt in wl_groups:
                    for j in range(ndk):
                        dk = dk0 + j
                        ks = slice(2 * dk, 2 * dk + 2)
                        js = slice(2 * j, 2 * j + 2)
                        nc.tensor.matmul(
                            acc[:], xh[:, ks, tsl], wht[:, js, :],
                            start=(dk == 0), stop=False, perf_mode=DR)
                        nc.tensor.matmul(
                            acc[:], xh[:, ks, tsl], wlt[:, js, :],
                            start=False, stop=False, perf_mode=DR)
                        nc.tensor.matmul(
                            acc[:], xl[:, ks, tsl], wht[:, js, :],
                            start=False, stop=False, perf_mode=DR)
                close_and_evac(acc, t, NB - 1, split_out=(t == TT - 1))

    nc.compile()
    return nc


def _get_nc():
    if "nc" not in _cache:
        _cache["nc"] = _build()
    return _cache["nc"]


def kernel(x, base_weight, lora_A, lora_B, bias, _trace=False,
           _trace_kwargs=None):
    import ml_dtypes
    from concourse.bass_utils import run_bass_kernel_spmd

    E4 = ml_dtypes.float8_e4m3

    def q8(a):
        return np.clip(a, -240.0, 240.0).astype(E4)

    nc = _get_nc()

    W = (np.asarray(base_weight, dtype=np.float32)
         + 2.0 * (np.asarray(lora_B, dtype=np.float32)
                  @ np.asarray(lora_A, dtype=np.float32)))
    # wt[k, o] = W[o, k], pre-tiled to [ob, p, c, o']
    ws = np.ascontiguousarray(W.T) * WSC
    ws_t = ws.reshape(KC, 128, NB, 512).transpose(2, 1, 0, 3)
    wh = q8(ws_t)
    wl = q8(ws_t - wh.astype(np.float32))
    wh = np.ascontiguousarray(wh)
    wl = np.ascontiguousarray(wl)

    bb = np.zeros((128, 2, O), dtype=E4)
    bb[0, 0, :] = q8(np.asarray(bias, dtype=np.float32) * BSC)

    x_flat = np.asarray(x, dtype=np.float32).reshape(T, D)
    xs_all = x_flat.T * XSC                       # [D, T]

    in_maps = []
    for c in range(T_SH):
        xs = xs_all[:, TC * c:TC * (c + 1)].reshape(KC, 128, TC)
        xs = np.ascontiguousarray(xs.transpose(1, 0, 2))  # [128, KC, TC]
        xhc = q8(xs)
        xlc = q8(xs - xhc.astype(np.float32))
        in_maps.append({"xh": xhc, "xl": xlc, "wh": wh, "wl": wl, "bb": bb})

    res = run_bass_kernel_spmd(nc, in_maps, list(range(8)),
                               trace=_trace, **(_trace_kwargs or {}))

    y = np.empty((T, O), dtype=np.float32)
    for c in range(T_SH):
        y[TC * c:TC * (c + 1), :] = res.results[c]["y"]
    out = y.reshape(x.shape[0], x.shape[1], O)
    if _trace:
        return out, res
    return out


# revision 6
# speedup vs baseline: 1.3763x; 1.3763x over previous
"""LoRALinear fused kernel for 8 trn2 NeuronCores — v5.

y = x @ (base + 2*(B@A))^T + bias,  x:[2,2048,4096], base:[4096,4096],
A:[8,4096], B:[4096,8], bias:[4096] -> y:[2,2048,4096], all fp32.

Sharding: 8-way token-parallel (512 tokens/core, replicated weights).

The GEMM wall on TRN2: the PE moving-operand feed is 2B/cycle/partition
in every <=16-bit mode (fp8 DoubleRow measured 216ns per K=256 MM — 2x
MACs/cycle but our 3-term hi/lo split needs 3x the MACs, netting 1.5x
SLOWER; see kernel_fp8_v4.py.bak).  So fp16 single-GEMM is optimal:
1024 MMs x 216ns = 221us/core.  v5 cuts the baseline's ~37us overhead
instead:

- LoRA folded into W on host (0.2% of the FLOPs): kills the 32 PT
  matmuls + at/ptw machinery (-7us PE).
- bias: one gpsimd partition_broadcast into a resident [128,4096] f32
  tile at the head; evac becomes DVE tensor_add(acc, bias_bc) instead
  of copy — kills the 32 bias-close matmuls (-7us PE).  stop=True
  rides the last k-chunk MM.
- W pre-tiled in DRAM as [ob, p, c, o] so group DMAs are 2-4KB/
  descriptor; x pre-tiled [p, c, t].  W groups + ob0's x groups
  alternate scalar/sync rings so neither ring exceeds ~150GB/s during
  o-block 0 (the baseline starved ~5us there on one ring).
- last o-block token-outer over resident W (prefetched during block 6)
  so the final drain is one tile, evac'd+DMA'd in 4 pipelined quarters.
"""
import sys

sys.path.insert(0, "/opt/trn_rl_repo")

import numpy as np

T_SH = 8                    # token shards (pure data-parallel)
T, D, O = 4096, 4096, 4096  # flattened tokens, d_in, d_out
TC = T // T_SH              # 512 tokens per core
KC = D // 128               # 32 contraction chunks
NB = O // 512               # 8 o-blocks of 512
TT = TC // 128              # 4 token tiles per core

_cache = {}


def _build():
    import concourse.mybir as mybir
    import concourse.tile as tile
    from concourse import bacc

    f32 = mybir.dt.float32
    fp16 = mybir.dt.float16

    nc = bacc.Bacc("TRN2", target_bir_lowering=False, debug=False,
                   num_devices=8)

    xt_d = nc.dram_tensor("xt", [128, KC, TC], fp16,
                          kind="ExternalInput").ap()
    wt_d = nc.dram_tensor("wt", [NB, 128, KC, 512], fp16,
                          kind="ExternalInput").ap()
    bias_d = nc.dram_tensor("bias", [1, O], fp16, kind="ExternalInput").ap()
    y_d = nc.dram_tensor("y", [TC, O], f32, kind="ExternalOutput").ap()

    # o-block DMA group layout in chunk units: small head groups for a
    # fast start + fine-grained ob0 pacing
    G_FIRST = [(0, 1), (1, 1), (2, 2), (4, 2), (6, 2), (8, 2), (10, 2),
               (12, 4), (16, 4), (20, 4), (24, 4), (28, 4)]
    G_STEADY = [(0, 2), (2, 2), (4, 4), (8, 4), (12, 4), (16, 4),
                (20, 4), (24, 4), (28, 4)]
    # x chunk-groups interleaved into ob0's W stream (chunk units)
    X_GROUPS = [(0, 2), (2, 4), (4, 8), (8, 12), (12, 17), (17, 22),
                (22, 27), (27, 32)]

    with tile.TileContext(nc) as tc:
        with (
            tc.tile_pool(name="res", bufs=1) as res,
            tc.tile_pool(name="wst", bufs=8) as wst,
            tc.tile_pool(name="evac", bufs=6) as evac,
            tc.tile_pool(name="psum", bufs=1, space="PSUM") as psum,
        ):
            xt = res.tile([128, KC, TC], fp16)
            # bias broadcast to all 128 partitions via a ones-row matmul in
            # the DMA-bound head window (PE and DVE are idle there anyway)
            ones_t = res.tile([128, 128], fp16)
            bb16 = res.tile([128, O], fp16)
            bias_bc = res.tile([128, O], f32)
            nc.vector.memset(ones_t[:], 0.0)
            nc.vector.memset(ones_t[0:1, :], 1.0)
            # rows 1-127 of bb16 get weight 0 in the matmul but must not be
            # NaN garbage, so memset the whole tile before loading row 0
            nc.vector.memset(bb16[:], 0.0)
            nc.scalar.dma_start(bb16[0:1, :], bias_d[:])
            for ob in range(NB):
                osl = slice(512 * ob, 512 * (ob + 1))
                pb = psum.tile([128, 512], f32, name=f"pbias{ob}",
                               tag=f"acc{ob % TT}", bufs=2)
                nc.tensor.matmul(pb[:], ones_t[:], bb16[:, osl],
                                 start=True, stop=True)
                nc.vector.tensor_copy(bias_bc[:, osl], pb[:])

            ring = [0]

            def next_ring():
                r = nc.sync if ring[0] % 2 == 0 else nc.scalar
                ring[0] += 1
                return r

            ev_ring = [0]

            def evac_out(acc, t, ob, split_out=False):
                osl = slice(512 * ob, 512 * (ob + 1))
                tsl = slice(128 * t, 128 * (t + 1))
                ev = evac.tile([128, 512], f32, name=f"ev{t}", tag="ev")
                if split_out:
                    # pipeline the tail: add+DMA in 128-col quarters
                    for q in range(4):
                        qs = slice(128 * q, 128 * (q + 1))
                        oq = slice(osl.start + 128 * q,
                                   osl.start + 128 * (q + 1))
                        nc.vector.tensor_add(ev[:, qs], acc[:, qs],
                                             bias_bc[:, oq])
                        rq = nc.scalar if q % 2 == 0 else nc.sync
                        rq.dma_start(y_d[tsl, oq], ev[:, qs])
                else:
                    nc.vector.tensor_add(ev[:], acc[:], bias_bc[:, osl])
                    r = nc.scalar if ev_ring[0] % 2 == 0 else nc.sync
                    ev_ring[0] += 1
                    r.dma_start(y_d[tsl, osl], ev[:])

            def dma_w_groups(ob, glist, interleave=None, tag_sfx=""):
                groups = []
                for g, (c0, ng) in enumerate(glist):
                    small = ng <= 2
                    wtile = wst.tile([128, ng, 512], fp16,
                                     name=f"wt{ob}_{g}",
                                     tag=("ws" if small else "wb") + tag_sfx,
                                     bufs=(6 if small and not tag_sfx
                                           else None))
                    nc_ring = next_ring()
                    nc_ring.dma_start(wtile[:], wt_d[ob][:, c0:c0 + ng, :])
                    groups.append((c0, ng, wtile))
                    if interleave is not None:
                        interleave(g)
                return groups

            def mm_block(accs, groups, ob):
                for c0, ng, wtile in groups:
                    for j in range(ng):
                        k = c0 + j
                        for t in range(TT):
                            nc.tensor.matmul(
                                accs[t][:],
                                xt[:, k, 128 * t:128 * (t + 1)],
                                wtile[:, j, :],
                                start=(k == 0), stop=(k == KC - 1))

            def x_filler(g):
                if g < len(X_GROUPS):
                    c0, c1 = X_GROUPS[g]
                    next_ring().dma_start(xt[:, c0:c1, :], xt_d[:, c0:c1, :])

            def o_block(ob, glist, interleave=None):
                accs = {
                    t: psum.tile([128, 512], f32, name=f"acc{t}_{ob}",
                                 tag=f"acc{t}", bufs=2)
                    for t in range(TT)
                }
                groups = dma_w_groups(ob, glist, interleave=interleave)
                mm_block(accs, groups, ob)
                for t in range(TT):
                    evac_out(accs[t], t, ob)

            o_block(0, G_FIRST, interleave=x_filler)
            for ob in range(1, NB - 2):
                o_block(ob, G_STEADY)

            # last o-block: token-outer over resident W (prefetched during
            # block NB-2) so the 4 closes stagger ~7us apart
            wl_groups = []

            def prefetch_wlast(g):
                if g < len(G_STEADY):
                    wl_groups.extend(
                        dma_w_groups(NB - 1, [G_STEADY[g]], tag_sfx="L"))

            o_block(NB - 2, G_STEADY, interleave=prefetch_wlast)

            for t in range(TT):
                acc = psum.tile([128, 512], f32, name=f"acc{t}_last",
                                tag=f"acc{t}", bufs=2)
                for c0, ng, wtile in wl_groups:
                    for j in range(ng):
                        k = c0 + j
                        nc.tensor.matmul(
                            acc[:], xt[:, k, 128 * t:128 * (t + 1)],
                            wtile[:, j, :],
                            start=(k == 0), stop=(k == KC - 1))
                evac_out(acc, t, NB - 1, split_out=(t == TT - 1))

    nc.compile()
    return nc


def _get_nc():
    if "nc" not in _cache:
        _cache["nc"] = _build()
    return _cache["nc"]


def kernel(x, base_weight, lora_A, lora_B, bias, _trace=False,
           _trace_kwargs=None):
    from concourse.bass_utils import run_bass_kernel_spmd

    nc = _get_nc()

    W = (np.asarray(base_weight, dtype=np.float32)
         + 2.0 * (np.asarray(lora_B, dtype=np.float32)
                  @ np.asarray(lora_A, dtype=np.float32)))
    # wt[k, o] = W[o, k], pre-tiled to [ob, p, c, o']
    wt = np.ascontiguousarray(
        W.T.reshape(KC, 128, NB, 512).transpose(2, 1, 0, 3)
    ).astype(np.float16)

    brow = np.asarray(bias, dtype=np.float32).reshape(1, O).astype(np.float16)

    x_flat = np.asarray(x, dtype=np.float32).reshape(T, D)
    xT = x_flat.T  # [D, T]

    in_maps = []
    for c in range(T_SH):
        xs = xT[:, TC * c:TC * (c + 1)].reshape(KC, 128, TC)
        xs = np.ascontiguousarray(xs.transpose(1, 0, 2)).astype(np.float16)
        in_maps.append({"xt": xs, "wt": wt, "bias": brow})

    res = run_bass_kernel_spmd(nc, in_maps, list(range(8)),
                               trace=_trace, **(_trace_kwargs or {}))

    y = np.empty((T, O), dtype=np.float32)
    for c in range(T_SH):
        y[TC * c:TC * (c + 1), :] = res.results[c]["y"]
    out = y.reshape(x.shape[0], x.shape[1], O)
    if _trace:
        return out, res
    return out
